# revision 30
# baseline (speedup 1.0000x reference)
"""Trainium2 Bass kernel for windowed multi-head attention with relative
position bias (Swin-style block):

    qkv = x @ qkv_w.T + [q_bias, 0, v_bias]
    q, k, v = split(qkv);  q *= hd**-0.5
    attn = softmax(q @ k.T + rel_table[rel_index])
    out  = (attn @ v) @ proj_w.T + proj_b

Shapes: x [8, 32, 32, 768], 12 heads, head_dim 64, N=1024 tokens.

Sharding: pure data-parallel - one batch element per NeuronCore, 8 cores,
no collectives.

Design notes (V18):
  - all matmuls fp16 (weights/x cast on host); psum accumulates f32.
  - pss is a ring of THREE [128,512] psum tiles (3 banks): S(kt+1) never
    waits the exp of kt (the V15-V17 ring-2 made the S<->exp latency
    chain the segment backbone at ~578ns/instr). exps then stream at
    ACT's ~585ns/[128,512] rate; PE streams independently.
  - the bank comes from the qk filler stream: single live pq tile
    (sequential qc chains), ps_w bufs=1. The pre-block V/qk0 pair-chains
    instead borrow a 2-buf [128,512] ring from the ps_o pool (tag "pre",
    temporally disjoint with the po ring).
  - denominator chains for heads 0..9: DVE copies the s row out of PSUM,
    DMA round-trip reshapes to [128,8], ln+exp(-x) on ACT (~190ns each),
    broadcast read, fused STT normalize; woven into segment h+2 at kt
    slots 0/3/5. Heads 10/11 use direct [1,512]-per-qc ln/exp chains at
    the tail (latency, not throughput, matters there).
  - gpsimd (Pool) takes bias-mult kts 0/1/2 (earliest-ready inputs);
    DVE takes kts 3..7 per qc-half. load_bias(h+2) at segment end so its
    biasT buffer WAR (DVE kt7 mult) can't convoy the Sync queue.
  - proj: k0..2 partial chains for ALL j stream as fillers from segment
    9 (their attn c-tiles normalized long before), evicted (+proj bias)
    into an fp16 SBUF acc; tail does per-(j,qc) psum groups
    [k3, k4, ident@acc, k5] - the identity matmul injects the partial so
    the final eviction is a plain psum copy, split ACT(qc0)/DVE(qc1).
  - y is written fp16 (host upcasts); halves the output flush that
    dominated the post-compute drain.
  - pre-block: DMA order xT -> wqk jj0/jj6 slices -> wv -> wqk rest ->
    biasT 0/1 (wp/pbias/ident deferred to segment 1); qk0 pair-chains
    emitted BEFORE the V chains so segment 0 is gated only by qk0.
  - PSUM: pss [128,512]x3 + po [65,2x512]x2 + pw [128,512]x1 = 8 banks
    (+2 "pre" banks inside ps_o's 4, pre-block only).
"""

import numpy as np

_CACHE = {}

B = 8
WS = 32
N = WS * WS            # 1024 tokens
C = 768
NH = 12
HD = 64
P = 128
QC = 2                 # q chunks of 512
QN = N // QC           # 512
KT = N // P            # 8 k tiles
CT = C // P            # 6 contraction tiles
OT_QK = (2 * C) // P   # 12 output tiles for q,k rows
VC = 2                 # v output chunks of 384
VN = C // VC           # 384
NP = NH // 2           # 6 head pairs
RT = N // P            # 8 = columns of the [128,8] reshaped denominators


def _build():
    import concourse.bass as bass
    import concourse.bacc as bacc
    import concourse.mybir as mybir
    import concourse.tile as tile
    import itertools
    from concourse.bass import _add_dep_helper

    f32 = mybir.dt.float32
    f16 = mybir.dt.float16
    AF = mybir.ActivationFunctionType
    MUL = mybir.AluOpType.mult

    nc = bacc.Bacc(None, target_bir_lowering=False)

    xT_d = nc.dram_tensor("xT", [C, N], f16, kind="ExternalInput")
    wqk_d = nc.dram_tensor("wqk", [C, 2 * C], f16, kind="ExternalInput")
    wv_d = nc.dram_tensor("wv", [C, C], f16, kind="ExternalInput")
    wp_d = nc.dram_tensor("wp", [C, C], f16, kind="ExternalInput")
    qkb_d = nc.dram_tensor("qkb", [OT_QK, P], f32, kind="ExternalInput")
    vb_d = nc.dram_tensor("vb", [C], f16, kind="ExternalInput")
    pb_d = nc.dram_tensor("pb", [CT, P], f32, kind="ExternalInput")
    biasT_d = nc.dram_tensor("biasT", [NH, N, N], f16, kind="ExternalInput")
    ident_d = nc.dram_tensor("ident", [P, P], f16, kind="ExternalInput")
    yT_d = nc.dram_tensor("yT", [C, N], f16, kind="ExternalOutput")
    s_d = nc.dram_tensor("s_scr", [NH, N], f16)
    inv_d = nc.dram_tensor("inv_scr", [NH, N], f16)

    with tile.TileContext(nc) as tc:
        with (
            tc.tile_pool(name="cst", bufs=1) as cst,
            tc.tile_pool(name="bias_pool", bufs=2) as bias_pool,
            tc.tile_pool(name="pt_pool", bufs=2) as pt_pool,
            tc.tile_pool(name="ps_s", bufs=3, space="PSUM") as ps_s,
            tc.tile_pool(name="ps_o", bufs=2, space="PSUM") as ps_o,
            tc.tile_pool(name="ps_w", bufs=1, space="PSUM") as ps_w,
        ):
            # ---- persistent SBUF ----
            q_t = cst.tile([P, CT, N], f16)          # Q^T  [c, t]
            k_pad = cst.tile([P, NH, N], f16)        # zero-padded K^T per head
            v_aug = cst.tile([P, KT, NH, HD + 1], f16)  # V + ones column
            attn = cst.tile([P, CT, N], f16)         # normalized attn out ^T
            acc = cst.tile([P, CT, N], f16)          # proj partial (k0..2)+pb
            xT = cst.tile([P, CT, N], f16)
            wv = cst.tile([P, CT, C], f16)
            wqk = cst.tile([P, CT, 2 * C], f16)
            wp = cst.tile([P, CT, C], f16)
            qkb = cst.tile([P, OT_QK], f32)
            vb_bc = cst.tile([P, C], f16)
            pbias = cst.tile([P, CT], f32)
            ident = cst.tile([P, P], f16)

            biasT = {}

            def load_bias(h):
                biasT[h] = bias_pool.tile([P, KT, N], f16, tag="biasT",
                                          name=f"biasT{h}")
                nc.sync.dma_start(
                    biasT[h], biasT_d[h].rearrange("(kt p) q -> p kt q", p=P))

            # ---- input DMAs, priority order: everything segment 0 needs
            # first (xT + the qk0 weight slices), then wv for the V
            # pre-block, then the rest. The DMA rings drain all queued
            # descriptors roughly concurrently, so keeping early bytes
            # small is what matters.
            xT_src = xT_d[:].rearrange("(k p) t -> p k t", p=P)
            wv_src = wv_d[:].rearrange("(k p) o -> p k o", p=P)
            wqk_src = wqk_d[:].rearrange("(k p) o -> p k o", p=P)
            wp_src = wp_d[:].rearrange("(k p) o -> p k o", p=P)
            for k in range(CT):
                nc.sync.dma_start(xT[:, k, :], xT_src[:, k, :])
                nc.sync.dma_start(wqk[:, k, 0:P], wqk_src[:, k, 0:P])
                nc.sync.dma_start(wqk[:, k, C:C + P], wqk_src[:, k, C:C + P])
            for k in range(CT):
                nc.sync.dma_start(wv[:, k, :], wv_src[:, k, :])
            nc.sync.dma_start(vb_bc, bass.AP(tensor=vb_d, offset=0,
                                             ap=[[0, P], [1, C]]))
            nc.sync.dma_start(qkb, qkb_d[:].rearrange("j p -> p j"))
            for k in range(CT):
                nc.sync.dma_start(wqk[:, k, P:C], wqk_src[:, k, P:C])
                nc.sync.dma_start(wqk[:, k, C + P:2 * C],
                                  wqk_src[:, k, C + P:2 * C])
            load_bias(0)
            load_bias(1)

            # preload the {ln, exp} activation table set once (set 6 =
            # natural_log_exp_and_others) so ln/exp never reload tables
            nc.scalar.add_instruction(mybir.InstLoadActFuncSet(
                name="preload_ln_exp", act_func_set_id=6, ins=[], outs=[]))

            # one-time fills: ones column on DVE (tiny; gpsimd memsets
            # take ~12us and the V evictions WAW-wait on them), k_pad
            # zeros on GPSIMD (first consumed by segment-0 S matmuls)
            nc.vector.memset(v_aug[:, :, :, HD:HD + 1], 1.0)
            nc.gpsimd.memset(k_pad[64:128, 0:NH:2, :], 0.0)
            nc.gpsimd.memset(k_pad[0:64, 1:NH:2, :], 0.0)

            # ---- pre-block: qk0 pair-chains FIRST (they alone gate
            # segment 0), then all of V; both on the ps_o "pre" ring ----
            def pre_pair():
                # pre-block pair-chains ride the pss ring-3 (2 live of 3;
                # pool sizing is static per tag, so no extra banks)
                return [ps_s.tile([P, QN], f32, tag="pss", name=f"pp{i}")
                        for i in range(2)]

            def qk0_pair(jj):
                pqs = pre_pair()
                for k in range(CT):
                    for qc in range(QC):
                        nc.tensor.matmul(
                            pqs[qc], wqk[:, k, jj * P:(jj + 1) * P],
                            xT[:, k, qc * QN:(qc + 1) * QN],
                            start=(k == 0), stop=(k == CT - 1))
                for qc in range(QC):
                    if jj < CT:
                        nc.vector.tensor_scalar_add(
                            q_t[:, jj, qc * QN:(qc + 1) * QN], pqs[qc],
                            qkb[:, jj:jj + 1])
                    else:
                        h0 = 2 * (jj - CT)
                        nc.vector.tensor_scalar_add(
                            k_pad[0:64, h0, qc * QN:(qc + 1) * QN],
                            pqs[qc][0:64, :], qkb[0:64, jj:jj + 1])
                        nc.vector.tensor_scalar_add(
                            k_pad[64:128, h0 + 1, qc * QN:(qc + 1) * QN],
                            pqs[qc][64:128, :], qkb[64:128, jj:jj + 1])

            def v_pair(tt):
                pvs = pre_pair()
                for k in range(CT):
                    for vc in range(VC):
                        nc.tensor.matmul(
                            pvs[vc][:, 0:VN], xT[:, k, tt * P:(tt + 1) * P],
                            wv[:, k, vc * VN:(vc + 1) * VN],
                            start=(k == 0), stop=(k == CT - 1))
                for vc in range(VC):
                    h0 = vc * (NH // VC)
                    nc.vector.tensor_add(
                        v_aug[:, tt, h0:h0 + NH // VC, 0:HD],
                        pvs[vc][:, 0:VN], vb_bc[:, vc * VN:(vc + 1) * VN])

            qk0_pair(0)
            qk0_pair(CT)
            for tt in range(KT):
                v_pair(tt)

            # ---- filler streams (single live pq tile on ps_w, one
            # matmul per unit, consumed 2 per kt slot) ----
            def qk_seq_units(j):
                for jj in (j, CT + j):
                    for qc in range(QC):
                        pq = ps_w.tile([P, QN], f32, tag="pw", name="pq")
                        for k in range(CT):
                            nc.tensor.matmul(
                                pq, wqk[:, k, jj * P:(jj + 1) * P],
                                xT[:, k, qc * QN:(qc + 1) * QN],
                                start=(k == 0), stop=(k == CT - 1))
                            yield
                        if jj < CT:
                            nc.vector.tensor_scalar_add(
                                q_t[:, jj, qc * QN:(qc + 1) * QN], pq,
                                qkb[:, jj:jj + 1])
                        else:
                            h0 = 2 * (jj - CT)
                            nc.vector.tensor_scalar_add(
                                k_pad[0:64, h0, qc * QN:(qc + 1) * QN],
                                pq[0:64, :], qkb[0:64, jj:jj + 1])
                            nc.vector.tensor_scalar_add(
                                k_pad[64:128, h0 + 1, qc * QN:(qc + 1) * QN],
                                pq[64:128, :], qkb[64:128, jj:jj + 1])
                        yield

            def proj_partial_units(j):
                # proj tile j partial: contraction k=0..2 only (ct0..2 are
                # normalized by segment 7), +pb on eviction into the fp16
                # SBUF acc. Tail groups add k3/k4/k5.
                for qc in range(QC):
                    py = ps_w.tile([P, QN], f32, tag="pw", name="py")
                    for k in range(3):
                        nc.tensor.matmul(
                            py, wp[:, k, j * P:(j + 1) * P],
                            attn[:, k, qc * QN:(qc + 1) * QN],
                            start=(k == 0), stop=(k == 2))
                        yield
                    nc.vector.tensor_scalar_add(
                        acc[:, j, qc * QN:(qc + 1) * QN], py,
                        pbias[:, j:j + 1])
                    yield

            # ---- denominator chain (heads 0..9), split in 3 parts so
            # the in-order ACT/DVE queues never block on DMA waits ----
            def chain_part1(h, po_t):
                s_sb = pt_pool.tile([1, QC, QN], f16, tag="s_sb", name="s_sb")
                nc.vector.tensor_scalar_add(s_sb, po_t[HD:HD + 1, :, :], 0.0)
                w1 = nc.sync.dma_start(s_d[h], s_sb)
                s128 = pt_pool.tile([P, RT], f16, tag="s128", name="s128")
                r1 = nc.sync.dma_start(
                    s128, bass.AP(tensor=s_d, offset=h * N,
                                  ap=[[RT, P], [1, RT]]))
                _add_dep_helper(r1.ins, w1.ins, sync=True, reason="s RAW")
                return s128

            def chain_part2(h, s128):
                nc.scalar.activation(s128, s128, AF.Ln, bias=0.0, scale=1.0)
                nc.scalar.activation(s128, s128, AF.Exp, bias=0.0, scale=-1.0)
                w2 = nc.sync.dma_start(
                    bass.AP(tensor=inv_d, offset=h * N,
                            ap=[[RT, P], [1, RT]]), s128)
                inv_bc = pt_pool.tile([HD, N], f16, tag="invbc", name="invbc")
                r2 = nc.sync.dma_start(
                    inv_bc, bass.AP(tensor=inv_d, offset=h * N,
                                    ap=[[0, HD], [1, N]]))
                _add_dep_helper(r2.ins, w2.ins, sync=True, reason="inv RAW")
                return inv_bc

            def chain_part3(h, po_t, inv_bc):
                pbase = (h % 2) * 64
                j = h // 2
                for qc in range(QC):
                    nc.vector.scalar_tensor_tensor(
                        attn[pbase:pbase + HD, j, qc * QN:(qc + 1) * QN],
                        po_t[0:HD, qc, :], 1.0,
                        inv_bc[0:HD, qc * QN:(qc + 1) * QN],
                        MUL, MUL)

            def emit_chain_direct(h, po_t):
                # tail chains: per-qc [1,512] ln/exp + one DMA round trip
                # each, pipelined, to halve the exposed latency
                for qc in range(QC):
                    inv_t = pt_pool.tile([1, QN], f16, tag="invt_d",
                                         name="inv_t")
                    nc.scalar.activation(inv_t, po_t[HD:HD + 1, qc, :],
                                         AF.Ln, bias=0.0, scale=1.0)
                    nc.scalar.activation(inv_t, inv_t,
                                         AF.Exp, bias=0.0, scale=-1.0)
                    off = h * N + qc * QN
                    w = nc.sync.dma_start(
                        bass.AP(tensor=inv_d, offset=off, ap=[[1, QN]]),
                        inv_t)
                    inv_bc = pt_pool.tile([HD, QN], f16, tag="invbc_d",
                                          name="invbc")
                    r = nc.sync.dma_start(
                        inv_bc, bass.AP(tensor=inv_d, offset=off,
                                        ap=[[0, HD], [1, QN]]))
                    _add_dep_helper(r.ins, w.ins, sync=True, reason="inv RAW")
                    pbase = (h % 2) * 64
                    j = h // 2
                    nc.vector.scalar_tensor_tensor(
                        attn[pbase:pbase + HD, j, qc * QN:(qc + 1) * QN],
                        po_t[0:HD, qc, :], 1.0, inv_bc[0:HD, :], MUL, MUL)

            # ---- sliding head pipeline ----
            stream_qk = itertools.chain(
                *[qk_seq_units(j) for j in range(1, NP)])
            stream_proj = itertools.chain(
                *[proj_partial_units(j) for j in range(CT)])
            po = {}
            pt = {}
            chain_state = {}
            for h in range(NH):
                pt[h] = pt_pool.tile([P, KT, N], f16, tag="pt",
                                     name=f"pt{h}")
                if h >= 1:
                    po[h - 1] = ps_o.tile([HD + 1, QC, QN], f32, tag="po",
                                          name=f"po{h - 1}")

                def pv_mms(hh, kt):
                    for qc in range(QC):
                        nc.tensor.matmul(
                            po[hh][:, qc, :], v_aug[:, kt, hh, :],
                            pt[hh][:, kt, qc * QN:(qc + 1) * QN],
                            start=(kt == 0), stop=(kt == KT - 1))

                for kt in range(KT):
                    for qc in range(QC):
                        psq = ps_s.tile([P, QN], f32, tag="pss", name="pss")
                        nc.tensor.matmul(
                            psq,
                            k_pad[:, h, kt * P:(kt + 1) * P],
                            q_t[:, h // 2, qc * QN:(qc + 1) * QN],
                            start=True, stop=True)
                        nc.scalar.activation(
                            pt[h][:, kt, qc * QN:(qc + 1) * QN], psq,
                            AF.Exp, bias=0.0, scale=1.0)
                    stream = stream_qk if h < 9 else stream_proj
                    next(stream, None)
                    next(stream, None)
                    # PV fillers staggered one k-tile behind the S stream
                    if h >= 1 and kt >= 1:
                        pv_mms(h - 1, kt - 1)
                    # gpsimd (Pool) takes kt 0/1/2: earliest-ready inputs,
                    # and biasT[h]'s last reader becomes the DVE kt7 mult
                    if kt in (0, 1, 2):
                        nc.gpsimd.tensor_mul(pt[h][:, kt, :], pt[h][:, kt, :],
                                             biasT[h][:, kt, :])
                    else:
                        for qc in range(QC):
                            nc.vector.tensor_mul(
                                pt[h][:, kt, qc * QN:(qc + 1) * QN],
                                pt[h][:, kt, qc * QN:(qc + 1) * QN],
                                biasT[h][:, kt, qc * QN:(qc + 1) * QN])
                    # weave the h-2 chain in after slots 0/3/5
                    if 2 <= h < NH:
                        if h - 2 <= 9:
                            if kt == 0:
                                chain_state[h - 2] = [
                                    chain_part1(h - 2, po[h - 2])]
                            elif kt == 3:
                                chain_state[h - 2].append(
                                    chain_part2(h - 2, chain_state[h - 2][0]))
                            elif kt == 5:
                                chain_part3(h - 2, po[h - 2],
                                            chain_state[h - 2][1])
                                del chain_state[h - 2]
                if h >= 1:
                    pv_mms(h - 1, KT - 1)
                # bias bulk DMA for h+2 at segment end: behind this
                # segment's chain DMAs on the Sync queue, and its buffer
                # WAR (DVE kt7 mult just emitted) resolves ~immediately
                if h + 2 < NH:
                    load_bias(h + 2)
                # wp & friends: deferred off the early DMA window
                if h == 1:
                    for k in range(CT):
                        nc.sync.dma_start(wp[:, k, :], wp_src[:, k, :])
                    nc.sync.dma_start(pbias, pb_d[:].rearrange("j p -> p j"))
                    nc.sync.dma_start(ident, ident_d[:])
                if h >= 2:
                    del po[h - 2], pt[h - 2]

            # ---- tail ----
            emit_chain_direct(NH - 2, po[NH - 2])
            po[NH - 1] = ps_o.tile([HD + 1, QC, QN], f32, tag="po",
                                   name=f"po{NH - 1}")
            for kt in range(KT):
                for qc in range(QC):
                    nc.tensor.matmul(
                        po[NH - 1][:, qc, :], v_aug[:, kt, NH - 1, :],
                        pt[NH - 1][:, kt, qc * QN:(qc + 1) * QN],
                        start=(kt == 0), stop=(kt == KT - 1))
            emit_chain_direct(NH - 1, po[NH - 1])

            # drain any leftover proj-partial units (attn k0..2 all ready)
            for _ in stream_proj:
                pass

            # per-(j,qc) groups [k3, k4, ident@acc, k5]: k3/k4/acc prerun
            # during the chain-11 latency where rings allow; k5 after the
            # STTs write attn ct5. Final evict = plain psum copy, split
            # ACT(qc0)/DVE(qc1), then fp16 y DMA.
            yT_dst = yT_d[:].rearrange("(j p) t -> p j t", p=P)
            for j in range(CT):
                pool, tg = (ps_w, "pw") if j % 2 == 0 else (ps_s, "pss")
                for qc in range(QC):
                    py = pool.tile([P, QN], f32, tag=tg, name="pyt")
                    for k in (3, 4):
                        nc.tensor.matmul(
                            py, wp[:, k, j * P:(j + 1) * P],
                            attn[:, k, qc * QN:(qc + 1) * QN],
                            start=(k == 3), stop=False)
                    nc.tensor.matmul(
                        py, ident[:, 0:P],
                        acc[:, j, qc * QN:(qc + 1) * QN],
                        start=False, stop=False)
                    nc.tensor.matmul(
                        py, wp[:, 5, j * P:(j + 1) * P],
                        attn[:, 5, qc * QN:(qc + 1) * QN],
                        start=False, stop=True)
                    yb = cst.tile([P, QN], f16, tag="yb", bufs=4, name="yb")
                    if qc == 0:
                        nc.scalar.activation(yb, py, AF.Copy,
                                             bias=0.0, scale=1.0)
                    else:
                        nc.vector.tensor_scalar_add(yb, py, 0.0)
                    nc.sync.dma_start(
                        yT_dst[:, j, qc * QN:(qc + 1) * QN], yb)

    nc.compile()
    return nc


def _get_nc():
    if "nc" not in _CACHE:
        _CACHE["nc"] = _build()
    return _CACHE["nc"]


def prepare_inputs(x, qkv_w, q_bias, v_bias, proj_w, proj_b, rel_table,
                   rel_index):
    """Host-side resharding/layout prep. Returns per-core input maps."""
    scale = HD ** -0.5
    x = np.asarray(x, np.float32)
    qkv_w = np.asarray(qkv_w, np.float32)
    q_bias = np.asarray(q_bias, np.float32)
    v_bias = np.asarray(v_bias, np.float32)
    proj_w = np.asarray(proj_w, np.float32)
    proj_b = np.asarray(proj_b, np.float32)
    rel_table = np.asarray(rel_table, np.float32)
    rel_index = np.asarray(rel_index)

    wq = qkv_w[0:C, :] * scale          # [o, c] rows scaled
    wk = qkv_w[C:2 * C, :]
    wv_ = qkv_w[2 * C:3 * C, :]
    wqk = np.ascontiguousarray(
        np.concatenate([wq, wk], axis=0).T).astype(np.float16)   # [c, 2C]
    wv_t = np.ascontiguousarray(wv_.T).astype(np.float16)        # [c, C]
    wp = np.ascontiguousarray(proj_w.T).astype(np.float16)       # [c, co]
    qkb = np.concatenate([q_bias * scale, np.zeros(C, np.float32)])
    qkb = np.ascontiguousarray(qkb.reshape(OT_QK, P))
    pb = np.ascontiguousarray(proj_b.reshape(CT, P))

    # bias[q, k, h] = rel_table[rel_index[q, k]]; ship exp(biasT[h, k, q])
    # so the kernel folds the softmax bias multiplicatively into P^T
    bias = rel_table[rel_index.reshape(-1)].reshape(N, N, NH)
    biasT = np.ascontiguousarray(
        np.exp(bias.transpose(2, 1, 0), dtype=np.float32)).astype(np.float16)

    shared = {
        "wqk": wqk, "wv": wv_t, "wp": wp, "qkb": qkb,
        "vb": v_bias.astype(np.float16), "pb": pb, "biasT": biasT,
        "ident": np.eye(P, dtype=np.float16),
    }
    in_maps = []
    for b in range(B):
        xt = np.ascontiguousarray(
            x[b].reshape(N, C).T).astype(np.float16)
        in_maps.append({"xT": xt, **shared})
    return in_maps


def kernel(x, qkv_w, q_bias, v_bias, proj_w, proj_b, rel_table, rel_index,
           _trace=False):
    from concourse.bass_utils import run_bass_kernel_spmd

    nc = _get_nc()
    in_maps = prepare_inputs(x, qkv_w, q_bias, v_bias, proj_w, proj_b,
                             rel_table, rel_index)
    kwargs = {}
    if _trace:
        import concourse.bass_utils as _bu
        _bu.upload_artifacts = lambda tmpdir: tmpdir
        kwargs = {"trace": True}
    res = run_bass_kernel_spmd(nc, in_maps, core_ids=list(range(B)), **kwargs)
    out = np.empty((B, WS, WS, C), np.float32)
    for b in range(B):
        out[b] = res.results[b]["yT"].astype(np.float32).T.reshape(WS, WS, C)
    if _trace:
        _CACHE["last_result"] = res
    return out


# revision 33
# speedup vs baseline: 1.1680x; 1.1680x over previous
"""Trainium2 Bass kernel for windowed multi-head attention with relative
position bias (Swin-style block):

    qkv = x @ qkv_w.T + [q_bias, 0, v_bias]
    q, k, v = split(qkv);  q *= hd**-0.5
    attn = softmax(q @ k.T + rel_table[rel_index])
    out  = (attn @ v) @ proj_w.T + proj_b

Shapes: x [8, 32, 32, 768], 12 heads, head_dim 64, N=1024 tokens.

Sharding: pure data-parallel - one batch element per NeuronCore, 8 cores,
no collectives.

Design notes (V18):
  - all matmuls fp16 (weights/x cast on host); psum accumulates f32.
  - pss is a ring of THREE [128,512] psum tiles (3 banks): S(kt+1) never
    waits the exp of kt (the V15-V17 ring-2 made the S<->exp latency
    chain the segment backbone at ~578ns/instr). exps then stream at
    ACT's ~585ns/[128,512] rate; PE streams independently.
  - the bank comes from the qk filler stream: single live pq tile
    (sequential qc chains), ps_w bufs=1. The pre-block V/qk0 pair-chains
    instead borrow a 2-buf [128,512] ring from the ps_o pool (tag "pre",
    temporally disjoint with the po ring).
  - denominator chains for heads 0..9: DVE copies the s row out of PSUM,
    DMA round-trip reshapes to [128,8], ln+exp(-x) on ACT (~190ns each),
    broadcast read, fused STT normalize; woven into segment h+2 at kt
    slots 0/3/5. Heads 10/11 use direct [1,512]-per-qc ln/exp chains at
    the tail (latency, not throughput, matters there).
  - gpsimd (Pool) takes bias-mult kts 0/1/2 (earliest-ready inputs);
    DVE takes kts 3..7 per qc-half. load_bias(h+2) at segment end so its
    biasT buffer WAR (DVE kt7 mult) can't convoy the Sync queue.
  - proj: k0..2 partial chains for ALL j stream as fillers from segment
    9 (their attn c-tiles normalized long before), evicted (+proj bias)
    into an fp16 SBUF acc; tail does per-(j,qc) psum groups
    [k3, k4, ident@acc, k5] - the identity matmul injects the partial so
    the final eviction is a plain psum copy, split ACT(qc0)/DVE(qc1).
  - y is written fp16 (host upcasts); halves the output flush that
    dominated the post-compute drain.
  - pre-block: DMA order xT -> wqk jj0/jj6 slices -> wv -> wqk rest ->
    biasT 0/1 (wp/pbias/ident deferred to segment 1); qk0 pair-chains
    emitted BEFORE the V chains so segment 0 is gated only by qk0.
  - PSUM: pss [128,512]x3 + po [65,2x512]x2 + pw [128,512]x1 = 8 banks
    (+2 "pre" banks inside ps_o's 4, pre-block only).
"""

import numpy as np

_CACHE = {}

B = 8
WS = 32
N = WS * WS            # 1024 tokens
C = 768
NH = 12
HD = 64
P = 128
QC = 2                 # q chunks of 512
QN = N // QC           # 512
KT = N // P            # 8 k tiles
CT = C // P            # 6 contraction tiles
OT_QK = (2 * C) // P   # 12 output tiles for q,k rows
VC = 2                 # v output chunks of 384
VN = C // VC           # 384
NP = NH // 2           # 6 head pairs
RT = N // P            # 8 = columns of the [128,8] reshaped denominators


def _build():
    import concourse.bass as bass
    import concourse.bacc as bacc
    import concourse.mybir as mybir
    import concourse.tile as tile
    import itertools
    from concourse.bass import _add_dep_helper

    f32 = mybir.dt.float32
    f16 = mybir.dt.float16
    AF = mybir.ActivationFunctionType
    MUL = mybir.AluOpType.mult

    nc = bacc.Bacc(None, target_bir_lowering=False)

    xT_d = nc.dram_tensor("xT", [C, N], f16, kind="ExternalInput")
    wqk_d = nc.dram_tensor("wqk", [C, 2 * C], f16, kind="ExternalInput")
    wv_d = nc.dram_tensor("wv", [C, C], f16, kind="ExternalInput")
    wp_d = nc.dram_tensor("wp", [C, C], f16, kind="ExternalInput")
    qkb_d = nc.dram_tensor("qkb", [OT_QK, P], f32, kind="ExternalInput")
    vb_d = nc.dram_tensor("vb", [C], f16, kind="ExternalInput")
    pb_d = nc.dram_tensor("pb", [CT, P], f32, kind="ExternalInput")
    biasT_d = nc.dram_tensor("biasT", [NH, N, N], f16, kind="ExternalInput")
    ident_d = nc.dram_tensor("ident", [P, P], f16, kind="ExternalInput")
    yT_d = nc.dram_tensor("yT", [C, N], f16, kind="ExternalOutput")
    s_d = nc.dram_tensor("s_scr", [NH, N], f16)
    inv_d = nc.dram_tensor("inv_scr", [NH, N], f16)

    with tile.TileContext(nc) as tc:
        with (
            tc.tile_pool(name="cst", bufs=1) as cst,
            tc.tile_pool(name="bias_pool", bufs=2) as bias_pool,
            tc.tile_pool(name="pt_pool", bufs=2) as pt_pool,
            tc.tile_pool(name="ps_s", bufs=3, space="PSUM") as ps_s,
            tc.tile_pool(name="ps_o", bufs=2, space="PSUM") as ps_o,
            tc.tile_pool(name="ps_w", bufs=1, space="PSUM") as ps_w,
        ):
            # ---- persistent SBUF ----
            q_t = cst.tile([P, CT, N], f16)          # Q^T  [c, t]
            k_pad = cst.tile([P, NH, N], f16)        # zero-padded K^T per head
            v_aug = cst.tile([P, KT, NH, HD + 1], f16)  # V + ones column
            attn = cst.tile([P, CT, N], f16)         # normalized attn out ^T
            acc = cst.tile([P, CT, N], f16)          # proj partial (k0..2)+pb
            xT = cst.tile([P, CT, N], f16)
            wv = cst.tile([P, CT, C], f16)
            wqk = cst.tile([P, CT, 2 * C], f16)
            wp = cst.tile([P, CT, C], f16)
            qkb = cst.tile([P, OT_QK], f32)
            vb_bc = cst.tile([P, C], f16)
            pbias = cst.tile([P, CT], f32)
            ident = cst.tile([P, P], f16)

            biasT = {}

            def load_bias(h):
                biasT[h] = bias_pool.tile([P, KT, N], f16, tag="biasT",
                                          name=f"biasT{h}")
                nc.sync.dma_start(
                    biasT[h], biasT_d[h].rearrange("(kt p) q -> p kt q", p=P))

            # ---- input DMAs, priority order: everything segment 0 needs
            # first (xT + the qk0 weight slices), then wv for the V
            # pre-block, then the rest. The DMA rings drain all queued
            # descriptors roughly concurrently, so keeping early bytes
            # small is what matters.
            xT_src = xT_d[:].rearrange("(k p) t -> p k t", p=P)
            wv_src = wv_d[:].rearrange("(k p) o -> p k o", p=P)
            wqk_src = wqk_d[:].rearrange("(k p) o -> p k o", p=P)
            wp_src = wp_d[:].rearrange("(k p) o -> p k o", p=P)
            for k in range(CT):
                nc.sync.dma_start(xT[:, k, :], xT_src[:, k, :])
                nc.sync.dma_start(wqk[:, k, 0:P], wqk_src[:, k, 0:P])
                nc.sync.dma_start(wqk[:, k, C:C + P], wqk_src[:, k, C:C + P])
            for k in range(CT):
                nc.sync.dma_start(wv[:, k, :], wv_src[:, k, :])
            nc.sync.dma_start(vb_bc, bass.AP(tensor=vb_d, offset=0,
                                             ap=[[0, P], [1, C]]))
            nc.sync.dma_start(qkb, qkb_d[:].rearrange("j p -> p j"))
            for k in range(CT):
                nc.sync.dma_start(wqk[:, k, P:C], wqk_src[:, k, P:C])
                nc.sync.dma_start(wqk[:, k, C + P:2 * C],
                                  wqk_src[:, k, C + P:2 * C])

            # preload the {ln, exp} activation table set once (set 6 =
            # natural_log_exp_and_others) so ln/exp never reload tables
            nc.scalar.add_instruction(mybir.InstLoadActFuncSet(
                name="preload_ln_exp", act_func_set_id=6, ins=[], outs=[]))

            # one-time fills: ones column on DVE (tiny; gpsimd memsets
            # take ~12us and the V evictions WAW-wait on them), k_pad
            # zeros on GPSIMD (first consumed by segment-0 S matmuls)
            nc.vector.memset(v_aug[:, :, :, HD:HD + 1], 1.0)
            nc.gpsimd.memset(k_pad[64:128, 0:NH:2, :], 0.0)
            nc.gpsimd.memset(k_pad[0:64, 1:NH:2, :], 0.0)

            # ---- pre-block: qk0 pair-chains FIRST (they alone gate
            # segment 0), then all of V; both on the ps_o "pre" ring ----
            def pre_pair():
                # pre-block pair-chains ride the pss ring-3 (2 live of 3;
                # pool sizing is static per tag, so no extra banks)
                return [ps_s.tile([P, QN], f32, tag="pss", name=f"pp{i}")
                        for i in range(2)]

            def qk0_pair(jj):
                pqs = pre_pair()
                for k in range(CT):
                    for qc in range(QC):
                        nc.tensor.matmul(
                            pqs[qc], wqk[:, k, jj * P:(jj + 1) * P],
                            xT[:, k, qc * QN:(qc + 1) * QN],
                            start=(k == 0), stop=(k == CT - 1))
                for qc in range(QC):
                    if jj < CT:
                        nc.vector.tensor_scalar_add(
                            q_t[:, jj, qc * QN:(qc + 1) * QN], pqs[qc],
                            qkb[:, jj:jj + 1])
                    else:
                        h0 = 2 * (jj - CT)
                        nc.vector.tensor_scalar_add(
                            k_pad[0:64, h0, qc * QN:(qc + 1) * QN],
                            pqs[qc][0:64, :], qkb[0:64, jj:jj + 1])
                        nc.vector.tensor_scalar_add(
                            k_pad[64:128, h0 + 1, qc * QN:(qc + 1) * QN],
                            pqs[qc][64:128, :], qkb[64:128, jj:jj + 1])

            def v_pair(tt):
                pvs = pre_pair()
                for k in range(CT):
                    for vc in range(VC):
                        nc.tensor.matmul(
                            pvs[vc][:, 0:VN], xT[:, k, tt * P:(tt + 1) * P],
                            wv[:, k, vc * VN:(vc + 1) * VN],
                            start=(k == 0), stop=(k == CT - 1))
                for vc in range(VC):
                    h0 = vc * (NH // VC)
                    nc.vector.tensor_add(
                        v_aug[:, tt, h0:h0 + NH // VC, 0:HD],
                        pvs[vc][:, 0:VN], vb_bc[:, vc * VN:(vc + 1) * VN])

            qk0_pair(0)
            qk0_pair(CT)
            for tt in range(KT):
                v_pair(tt)
            # bias 0/1 loads emitted after the pre-block matmuls so their
            # 4.2MB doesn't share the DMA rings with the pre-block weights
            load_bias(0)
            load_bias(1)

            # ---- filler streams (single live pq tile on ps_w, one
            # matmul per unit, consumed 2 per kt slot) ----
            def qk_seq_units(j):
                for jj in (j, CT + j):
                    for qc in range(QC):
                        pq = ps_w.tile([P, QN], f32, tag="pw", name="pq")
                        for k in range(CT):
                            nc.tensor.matmul(
                                pq, wqk[:, k, jj * P:(jj + 1) * P],
                                xT[:, k, qc * QN:(qc + 1) * QN],
                                start=(k == 0), stop=(k == CT - 1))
                            yield
                        if jj < CT:
                            nc.vector.tensor_scalar_add(
                                q_t[:, jj, qc * QN:(qc + 1) * QN], pq,
                                qkb[:, jj:jj + 1])
                        else:
                            h0 = 2 * (jj - CT)
                            nc.vector.tensor_scalar_add(
                                k_pad[0:64, h0, qc * QN:(qc + 1) * QN],
                                pq[0:64, :], qkb[0:64, jj:jj + 1])
                            nc.vector.tensor_scalar_add(
                                k_pad[64:128, h0 + 1, qc * QN:(qc + 1) * QN],
                                pq[64:128, :], qkb[64:128, jj:jj + 1])
                        yield

            def proj_partial_units(j):
                # proj tile j partial: contraction k=0..2 only (ct0..2 are
                # normalized by segment 7), +pb on eviction into the fp16
                # SBUF acc. Tail groups add k3/k4/k5.
                for qc in range(QC):
                    py = ps_w.tile([P, QN], f32, tag="pw", name="py")
                    for k in range(3):
                        nc.tensor.matmul(
                            py, wp[:, k, j * P:(j + 1) * P],
                            attn[:, k, qc * QN:(qc + 1) * QN],
                            start=(k == 0), stop=(k == 2))
                        yield
                    nc.vector.tensor_scalar_add(
                        acc[:, j, qc * QN:(qc + 1) * QN], py,
                        pbias[:, j:j + 1])
                    yield

            # ---- denominator chain (heads 0..9), split in 3 parts so
            # the in-order ACT/DVE queues never block on DMA waits ----
            def chain_part1(h, po_t):
                s_sb = pt_pool.tile([1, QC, QN], f16, tag="s_sb", name="s_sb")
                nc.vector.tensor_scalar_add(s_sb, po_t[HD:HD + 1, :, :], 0.0)
                w1 = nc.sync.dma_start(s_d[h], s_sb)
                s128 = pt_pool.tile([P, RT], f16, tag="s128", name="s128")
                r1 = nc.sync.dma_start(
                    s128, bass.AP(tensor=s_d, offset=h * N,
                                  ap=[[RT, P], [1, RT]]))
                _add_dep_helper(r1.ins, w1.ins, sync=True, reason="s RAW")
                return s128

            def chain_part2(h, s128):
                nc.scalar.activation(s128, s128, AF.Ln, bias=0.0, scale=1.0)
                nc.scalar.activation(s128, s128, AF.Exp, bias=0.0, scale=-1.0)
                w2 = nc.sync.dma_start(
                    bass.AP(tensor=inv_d, offset=h * N,
                            ap=[[RT, P], [1, RT]]), s128)
                inv_bc = pt_pool.tile([HD, N], f16, tag="invbc", name="invbc")
                r2 = nc.sync.dma_start(
                    inv_bc, bass.AP(tensor=inv_d, offset=h * N,
                                    ap=[[0, HD], [1, N]]))
                _add_dep_helper(r2.ins, w2.ins, sync=True, reason="inv RAW")
                return inv_bc

            def chain_part3(h, po_t, inv_bc):
                pbase = (h % 2) * 64
                j = h // 2
                for qc in range(QC):
                    nc.vector.scalar_tensor_tensor(
                        attn[pbase:pbase + HD, j, qc * QN:(qc + 1) * QN],
                        po_t[0:HD, qc, :], 1.0,
                        inv_bc[0:HD, qc * QN:(qc + 1) * QN],
                        MUL, MUL)

            def emit_chain_direct(h, po_t):
                # tail chains: per-qc [1,512] ln/exp + one DMA round trip
                # each, pipelined, to halve the exposed latency
                for qc in range(QC):
                    inv_t = pt_pool.tile([1, QN], f16, tag="invt_d",
                                         name="inv_t")
                    nc.scalar.activation(inv_t, po_t[HD:HD + 1, qc, :],
                                         AF.Ln, bias=0.0, scale=1.0)
                    nc.scalar.activation(inv_t, inv_t,
                                         AF.Exp, bias=0.0, scale=-1.0)
                    off = h * N + qc * QN
                    w = nc.sync.dma_start(
                        bass.AP(tensor=inv_d, offset=off, ap=[[1, QN]]),
                        inv_t)
                    inv_bc = pt_pool.tile([HD, QN], f16, tag="invbc_d",
                                          name="invbc")
                    r = nc.sync.dma_start(
                        inv_bc, bass.AP(tensor=inv_d, offset=off,
                                        ap=[[0, HD], [1, QN]]))
                    _add_dep_helper(r.ins, w.ins, sync=True, reason="inv RAW")
                    pbase = (h % 2) * 64
                    j = h // 2
                    nc.vector.scalar_tensor_tensor(
                        attn[pbase:pbase + HD, j, qc * QN:(qc + 1) * QN],
                        po_t[0:HD, qc, :], 1.0, inv_bc[0:HD, :], MUL, MUL)

            # ---- sliding head pipeline ----
            stream_qk = itertools.chain(
                *[qk_seq_units(j) for j in range(1, NP)])
            stream_proj = itertools.chain(
                *[proj_partial_units(j) for j in range(CT)])
            po = {}
            pt = {}
            chain_state = {}
            for h in range(NH):
                pt[h] = pt_pool.tile([P, KT, N], f16, tag="pt",
                                     name=f"pt{h}")
                if h >= 1:
                    po[h - 1] = ps_o.tile([HD + 1, QC, QN], f32, tag="po",
                                          name=f"po{h - 1}")

                def pv_mms(hh, kt):
                    for qc in range(QC):
                        nc.tensor.matmul(
                            po[hh][:, qc, :], v_aug[:, kt, hh, :],
                            pt[hh][:, kt, qc * QN:(qc + 1) * QN],
                            start=(kt == 0), stop=(kt == KT - 1))

                for kt in range(KT):
                    for qc in range(QC):
                        psq = ps_s.tile([P, QN], f32, tag="pss", name="pss")
                        nc.tensor.matmul(
                            psq,
                            k_pad[:, h, kt * P:(kt + 1) * P],
                            q_t[:, h // 2, qc * QN:(qc + 1) * QN],
                            start=True, stop=True)
                        nc.scalar.activation(
                            pt[h][:, kt, qc * QN:(qc + 1) * QN], psq,
                            AF.Exp, bias=0.0, scale=1.0)
                    stream = stream_qk if h < 9 else stream_proj
                    next(stream, None)
                    next(stream, None)
                    # PV fillers staggered one k-tile behind the S stream
                    if h >= 1 and kt >= 1:
                        pv_mms(h - 1, kt - 1)
                    # gpsimd (Pool) takes kt 0/1: earliest-ready inputs,
                    # and biasT[h]'s last reader becomes the DVE kt7 mult
                    if kt in (0, 1):
                        nc.gpsimd.tensor_mul(pt[h][:, kt, :], pt[h][:, kt, :],
                                             biasT[h][:, kt, :])
                    else:
                        for qc in range(QC):
                            nc.vector.tensor_mul(
                                pt[h][:, kt, qc * QN:(qc + 1) * QN],
                                pt[h][:, kt, qc * QN:(qc + 1) * QN],
                                biasT[h][:, kt, qc * QN:(qc + 1) * QN])
                    # weave the h-2 chain in after slots 0/3/5
                    if 2 <= h < NH:
                        if h - 2 <= 9:
                            if kt == 0:
                                chain_state[h - 2] = [
                                    chain_part1(h - 2, po[h - 2])]
                            elif kt == 3:
                                chain_state[h - 2].append(
                                    chain_part2(h - 2, chain_state[h - 2][0]))
                            elif kt == 5:
                                chain_part3(h - 2, po[h - 2],
                                            chain_state[h - 2][1])
                                del chain_state[h - 2]
                if h >= 1:
                    pv_mms(h - 1, KT - 1)
                # bias bulk DMA for h+2 at segment end: behind this
                # segment's chain DMAs on the Sync queue, and its buffer
                # WAR (DVE kt7 mult just emitted) resolves ~immediately
                if h + 2 < NH:
                    load_bias(h + 2)
                # wp & friends: deferred off the early DMA window
                if h == 1:
                    for k in range(CT):
                        nc.sync.dma_start(wp[:, k, :], wp_src[:, k, :])
                    nc.sync.dma_start(pbias, pb_d[:].rearrange("j p -> p j"))
                    nc.sync.dma_start(ident, ident_d[:])
                if h >= 2:
                    del po[h - 2], pt[h - 2]

            # ---- tail ----
            emit_chain_direct(NH - 2, po[NH - 2])
            po[NH - 1] = ps_o.tile([HD + 1, QC, QN], f32, tag="po",
                                   name=f"po{NH - 1}")
            for kt in range(KT):
                for qc in range(QC):
                    nc.tensor.matmul(
                        po[NH - 1][:, qc, :], v_aug[:, kt, NH - 1, :],
                        pt[NH - 1][:, kt, qc * QN:(qc + 1) * QN],
                        start=(kt == 0), stop=(kt == KT - 1))
            emit_chain_direct(NH - 1, po[NH - 1])

            # drain any leftover proj-partial units (attn k0..2 all ready)
            for _ in stream_proj:
                pass

            # per-(j,qc) groups [k3, k4, ident@acc, k5]: k3/k4/acc prerun
            # during the chain-11 latency where rings allow; k5 after the
            # STTs write attn ct5. Final evict = plain psum copy, split
            # ACT(qc0)/DVE(qc1), then fp16 y DMA.
            yT_dst = yT_d[:].rearrange("(j p) t -> p j t", p=P)
            for j in range(CT):
                pool, tg = (ps_w, "pw") if j % 2 == 0 else (ps_s, "pss")
                for qc in range(QC):
                    py = pool.tile([P, QN], f32, tag=tg, name="pyt")
                    for k in (3, 4):
                        nc.tensor.matmul(
                            py, wp[:, k, j * P:(j + 1) * P],
                            attn[:, k, qc * QN:(qc + 1) * QN],
                            start=(k == 3), stop=False)
                    nc.tensor.matmul(
                        py, ident[:, 0:P],
                        acc[:, j, qc * QN:(qc + 1) * QN],
                        start=False, stop=False)
                    nc.tensor.matmul(
                        py, wp[:, 5, j * P:(j + 1) * P],
                        attn[:, 5, qc * QN:(qc + 1) * QN],
                        start=False, stop=True)
                    yb = cst.tile([P, QN], f16, tag="yb", bufs=4, name="yb")
                    if qc == 0:
                        nc.scalar.activation(yb, py, AF.Copy,
                                             bias=0.0, scale=1.0)
                    else:
                        nc.vector.tensor_scalar_add(yb, py, 0.0)
                    nc.sync.dma_start(
                        yT_dst[:, j, qc * QN:(qc + 1) * QN], yb)

    nc.compile()
    return nc


def _get_nc():
    if "nc" not in _CACHE:
        _CACHE["nc"] = _build()
    return _CACHE["nc"]


def prepare_inputs(x, qkv_w, q_bias, v_bias, proj_w, proj_b, rel_table,
                   rel_index):
    """Host-side resharding/layout prep. Returns per-core input maps."""
    scale = HD ** -0.5
    x = np.asarray(x, np.float32)
    qkv_w = np.asarray(qkv_w, np.float32)
    q_bias = np.asarray(q_bias, np.float32)
    v_bias = np.asarray(v_bias, np.float32)
    proj_w = np.asarray(proj_w, np.float32)
    proj_b = np.asarray(proj_b, np.float32)
    rel_table = np.asarray(rel_table, np.float32)
    rel_index = np.asarray(rel_index)

    wq = qkv_w[0:C, :] * scale          # [o, c] rows scaled
    wk = qkv_w[C:2 * C, :]
    wv_ = qkv_w[2 * C:3 * C, :]
    wqk = np.ascontiguousarray(
        np.concatenate([wq, wk], axis=0).T).astype(np.float16)   # [c, 2C]
    wv_t = np.ascontiguousarray(wv_.T).astype(np.float16)        # [c, C]
    wp = np.ascontiguousarray(proj_w.T).astype(np.float16)       # [c, co]
    qkb = np.concatenate([q_bias * scale, np.zeros(C, np.float32)])
    qkb = np.ascontiguousarray(qkb.reshape(OT_QK, P))
    pb = np.ascontiguousarray(proj_b.reshape(CT, P))

    # bias[q, k, h] = rel_table[rel_index[q, k]]; ship exp(biasT[h, k, q])
    # so the kernel folds the softmax bias multiplicatively into P^T
    bias = rel_table[rel_index.reshape(-1)].reshape(N, N, NH)
    biasT = np.ascontiguousarray(
        np.exp(bias.transpose(2, 1, 0), dtype=np.float32)).astype(np.float16)

    shared = {
        "wqk": wqk, "wv": wv_t, "wp": wp, "qkb": qkb,
        "vb": v_bias.astype(np.float16), "pb": pb, "biasT": biasT,
        "ident": np.eye(P, dtype=np.float16),
    }
    in_maps = []
    for b in range(B):
        xt = np.ascontiguousarray(
            x[b].reshape(N, C).T).astype(np.float16)
        in_maps.append({"xT": xt, **shared})
    return in_maps


def kernel(x, qkv_w, q_bias, v_bias, proj_w, proj_b, rel_table, rel_index,
           _trace=False):
    from concourse.bass_utils import run_bass_kernel_spmd

    nc = _get_nc()
    in_maps = prepare_inputs(x, qkv_w, q_bias, v_bias, proj_w, proj_b,
                             rel_table, rel_index)
    kwargs = {}
    if _trace:
        import concourse.bass_utils as _bu
        _bu.upload_artifacts = lambda tmpdir: tmpdir
        kwargs = {"trace": True}
    res = run_bass_kernel_spmd(nc, in_maps, core_ids=list(range(B)), **kwargs)
    out = np.empty((B, WS, WS, C), np.float32)
    for b in range(B):
        out[b] = res.results[b]["yT"].astype(np.float32).T.reshape(WS, WS, C)
    if _trace:
        _CACHE["last_result"] = res
    return out
